# revision 48
# baseline (speedup 1.0000x reference)
"""Segment-mean (CGCNN crystal pooling) Bass kernel for 8 Trainium2 NeuronCores.

Reference computes, for sorted segment_ids over 1M atoms with 128 features:
    out[s] = sum(atom_fea[segment_ids == s]) / max(count(s), 1)   s in [0, 16384)

Strategy (data-parallel over crystals, no cross-device communication):
  - Core c owns segments [2048*c, 2048*(c+1)) = G groups of W segments.
  - Host pads each group's atoms to a uniform budget T*128 and lays them out
    partition-major: column block t of fea[g] ([128, T*128]) holds atom tile t
    ([128 atoms in partitions] x [128 features]).
  - Features ship as a SINGLE bf16 stream (harness gate is rel_err < 2e-2;
    bf16 rounding of the inputs gives ~2e-3 through the mean) -> HBM traffic
    is halved vs an exact hi/lo pair.
  - Narrow-window matmuls: segment_ids are sorted, so the 128 atoms of tile t
    only span a few segments. b(t) = min over ALL 128 groups (8 cores x G --
    the SPMD program is shared) of the group-relative id at atom position
    128*t, clamped to W-S; S = max span observed. Tile t's matmul then writes
    only psum[:, b(t):b(t)+S] with a [128, S] one-hot slice as the moving
    operand. A single full-width zero matmul per group initializes PSUM.
  - The one-hot block [128, T*S] is ONE DVE is_equal per group: a host-shipped
    tiled int8 iota constant vs a stride-0 broadcast of per-atom int8 ids
    relative to b(t) (padding atoms carry -1 and zero features). ~12x less
    DVE work than a full-width [128, T*W] compare.
  - Divide-by-count on device: a K=1 matmul broadcasts the group's 1/count
    row (f32) into all 128 PSUM partitions, ACT stages it to SBUF, and the
    eviction is a DVE multiply psum * invc -> bf16 out (the harness gate is
    2e-2; bf16 out adds ~1e-3).

Measured on trn2 (8 cores, axon): ~90-104 us/kernel (best 89.8, session
noise is +-8%) against a ~96 us DMA wire floor (34.6 MB/core at 16 engines
x 22.5 B/ns nominal; good sessions sustain ~385 GB/s/core). Baseline (exact
bf16 hi/lo pair, full-width one-hot, f32 out): ~216 us. Max relative error
vs the f32 reference: 3.1e-03 (gate: 2e-2).
"""

import contextlib

import ml_dtypes
import numpy as np

import concourse.bass as bass
import concourse.tile as tile
from concourse import bacc, mybir
from concourse.bass_utils import run_bass_kernel_spmd

try:
    import jax
    from jax.experimental.shard_map import shard_map
    from jax.sharding import Mesh, NamedSharding, PartitionSpec
    from concourse.bass2jax import (_bass_exec_p, install_neuronx_cc_hook,
                                    partition_id_tensor)
    _HAVE_FAST_PATH = True
except Exception:  # pragma: no cover - fall back to run_bass_kernel_spmd
    _HAVE_FAST_PATH = False

N = 1048576
FEA = 128
N0 = 16384
NCORES = 8
W = 512                     # segments per group (PSUM free dim = full bank)
SEGS_PER_CORE = N0 // NCORES  # 2048
G = SEGS_PER_CORE // W      # groups per core
NGROUPS = N0 // W           # groups total (all cores share one SPMD program)
P = 128
SB = 37                     # atom tiles per fea DMA block
FEA_BUFS = 5
DUAL_RING = True            # alternate fea slabs between sync and DVE rings
BF16 = ml_dtypes.bfloat16

_prog_cache: dict = {}


def build_program(plan, loop_repeat: int = 1):
    """SPMD Tile program for plan = (T, S, bases).

    T atom-tiles (T*128 atoms) per group; tile t's matmul writes the S-wide
    psum window starting at compile-time base bases[t]. loop_repeat > 1 wraps
    the body in a hardware For_i loop (timing only)."""
    T, S, bases = plan
    key = (T, S, bases, loop_repeat, SB, FEA_BUFS, DUAL_RING)
    if key in _prog_cache:
        return _prog_cache[key]

    f32 = mybir.dt.float32
    bf16 = mybir.dt.bfloat16
    nc = bacc.Bacc("TRN2", target_bir_lowering=False, debug=False,
                   num_devices=NCORES)
    i8 = mybir.dt.int8
    fhi = nc.dram_tensor("fhi", [G, P, T * P], bf16, kind="ExternalInput").ap()
    idsr = nc.dram_tensor("idsr", [P, G * T], i8, kind="ExternalInput").ap()
    iotar = nc.dram_tensor("iotar", [1, T * S], i8, kind="ExternalInput").ap()
    invc = nc.dram_tensor("invc", [1, G * W], f32, kind="ExternalInput").ap()
    out = nc.dram_tensor("out", [G, P, W], bf16, kind="ExternalOutput").ap()

    sb = min(T, SB)
    blocks = [(s, min(s + sb, T)) for s in range(0, T, sb)]

    with tile.TileContext(nc) as tc:
        with (
            tc.tile_pool(name="const", bufs=1) as const_pool,
            tc.tile_pool(name="fea", bufs=FEA_BUFS) as fea_pool,
            tc.tile_pool(name="meta", bufs=3) as meta_pool,
            tc.tile_pool(name="oh", bufs=2) as oh_pool,
            tc.tile_pool(name="evict", bufs=2) as evict_pool,
            tc.tile_pool(name="psum", bufs=2, space="PSUM") as psum_pool,
            tc.tile_pool(name="invp", bufs=2, space="PSUM") as invp_pool,
        ):
            # one 4.6KB DRAM row replicated across all 128 partitions
            iota_rep = const_pool.tile([P, T * S], i8)
            nc.scalar.dma_start(iota_rep[:], iotar.to_broadcast([P, T * S]))
            zeros = const_pool.tile([P, max(W, P)], bf16)
            nc.vector.memset(zeros[:], 0.0)
            ones = const_pool.tile([1, P], f32)
            nc.vector.memset(ones[:], 1.0)

            loop_ctx = (tc.For_i(0, loop_repeat, 1) if loop_repeat > 1
                        else contextlib.nullcontext())
            with loop_ctx:
                # all groups' meta in ONE wide DMA each (>=1KB rows dodge the
                # sub-512B small-transfer penalty); bulk fea on the sync queue
                ids_all = meta_pool.tile([P, G * T], i8, tag="ids")
                nc.scalar.dma_start(ids_all[:], idsr)
                invc_all = meta_pool.tile([1, G * W], f32, tag="invc")
                nc.scalar.dma_start(invc_all[:], invc)
                for g in range(G):
                    # K=1 matmul broadcasts 1/count into all 128 partitions
                    # (then ACT stages it to SBUF: DVE has one PSUM read port)
                    invp = invp_pool.tile([P, W], f32)
                    nc.tensor.matmul(out=invp[:], lhsT=ones[:],
                                     rhs=invc_all[:, g * W:(g + 1) * W],
                                     start=True, stop=True)
                    invp_sb = meta_pool.tile([P, W], f32)
                    nc.scalar.copy(invp_sb[:], invp[:])
                    oh_blk = oh_pool.tile([P, T * S], bf16)
                    nc.vector.tensor_tensor(
                        out=oh_blk[:], in0=iota_rep[:],
                        in1=ids_all[:, g * T:(g + 1) * T].to_broadcast(
                            [P, T, S]),
                        op=mybir.AluOpType.is_equal)
                    psum = psum_pool.tile([P, W], f32)
                    nc.tensor.matmul(
                        out=psum[:], lhsT=zeros[:, :P], rhs=zeros[:, :W],
                        start=True, stop=False)
                    for bi, (s, e) in enumerate(blocks):
                        hi_sb = fea_pool.tile([P, sb * P], bf16, tag="hi")
                        eng = (nc.scalar if (DUAL_RING and bi % 2)
                               else nc.sync)
                        eng.dma_start(hi_sb[:, :(e - s) * P],
                                      fhi[g][:, s * P:e * P])
                        for t in range(s, e):
                            c0 = (t - s) * P
                            b = bases[t]
                            nc.tensor.matmul(
                                out=psum[:, b:b + S],
                                lhsT=hi_sb[:, c0:c0 + P],
                                rhs=oh_blk[:, t * S:(t + 1) * S],
                                start=False, stop=(t == T - 1))
                    out_sb = evict_pool.tile([P, W], bf16)
                    nc.vector.tensor_tensor(out=out_sb[:], in0=psum[:],
                                            in1=invp_sb[:],
                                            op=mybir.AluOpType.mult)
                    nc.scalar.dma_start(out[g], out_sb[:])
    nc.compile()
    _prog_cache[key] = nc
    return nc


def prepare_inputs(atom_fea: np.ndarray, segment_ids: np.ndarray):
    """Shard + pad + layout inputs for the 8 cores. Returns (in_maps, plan)."""
    atom_fea = np.ascontiguousarray(atom_fea, dtype=np.float32)
    segment_ids = np.ascontiguousarray(segment_ids, dtype=np.int32)

    counts = np.bincount(segment_ids, minlength=N0)
    inv_counts = (1.0 / np.maximum(counts, 1)).astype(np.float32)

    bounds = np.searchsorted(segment_ids, np.arange(0, N0 + 1, W))
    ng = np.diff(bounds)
    T = max(1, int(np.ceil(ng.max() / P)))

    # Narrow-window plan shared by the single SPMD program: for tile t,
    # b(t) = min over all groups of the group-relative id of atom 128*t,
    # S = max observed span (id - b + 1) within any tile.
    lo_t = np.full(T, np.iinfo(np.int64).max, dtype=np.int64)
    hi_t = np.full(T, -1, dtype=np.int64)
    rel_groups = []
    for gi in range(NGROUPS):
        a = segment_ids[bounds[gi]:bounds[gi + 1]].astype(np.int64) - W * gi
        rel_groups.append(a)
        nt = int(np.ceil(len(a) / P))
        for t in range(nt):
            seg = a[t * P:(t + 1) * P]
            lo_t[t] = min(lo_t[t], seg[0])
            hi_t[t] = max(hi_t[t], seg[-1])
    S = int((hi_t - np.minimum(lo_t, hi_t + 1) + 1).max())
    bases = np.minimum(np.minimum(lo_t, W - S), hi_t + 1)
    bases = np.maximum(bases, 0)
    # guarantee every tile's ids fall inside [b, b+S) for every group
    assert int((hi_t - bases + 1).max()) <= S
    plan = (T, S, tuple(int(x) for x in bases))

    hi_full = atom_fea.astype(BF16)

    iota_rep = np.tile(np.arange(S, dtype=np.int8), T).reshape(1, T * S)

    in_maps = []
    for c in range(NCORES):
        hi_c = np.zeros((G, P, T * P), dtype=BF16)
        ids_c = np.empty((P, G * T), dtype=np.int8)
        for g in range(G):
            gidx = c * G + g
            lo_i, hi_i = bounds[gidx], bounds[gidx + 1]
            n = hi_i - lo_i
            blk = np.zeros((T * P, FEA), dtype=BF16)
            blk[:n] = hi_full[lo_i:hi_i]
            hi_c[g] = blk.reshape(T, P, FEA).transpose(1, 0, 2).reshape(
                P, T * P)
            idb = np.full(T * P, -1, dtype=np.int64)
            idb[:n] = rel_groups[gidx] - np.repeat(bases, P)[:n]
            ids_c[:, g * T:(g + 1) * T] = idb.reshape(T, P).T.astype(np.int8)
        invc_c = inv_counts[c * SEGS_PER_CORE:(c + 1) * SEGS_PER_CORE].reshape(
            1, G * W)
        in_maps.append({"fhi": hi_c, "idsr": ids_c, "iotar": iota_rep,
                        "invc": invc_c})
    return in_maps, plan


def assemble_output(results) -> np.ndarray:
    """[ncores][G, 128 fea, W seg] -> (N0, FEA)."""
    stacked = np.stack([results[c]["out"] for c in range(NCORES)]).astype(
        np.float32)
    return np.ascontiguousarray(
        stacked.transpose(0, 1, 3, 2).reshape(N0, FEA))


def _run_spmd_fast(nc, in_maps):
    """Execute the SPMD program on cores 0-7 via PJRT with explicit sharded
    device_put (same _bass_exec_p mechanism run_bass_kernel_spmd uses under
    axon, minus its per-call retrace and slow implicit transfers)."""
    install_neuronx_cc_hook()
    partition_name = (nc.partition_id_tensor.name
                      if nc.partition_id_tensor else None)
    in_names, out_names, out_avals = [], [], []
    for alloc in nc.m.functions[0].allocations:
        if not isinstance(alloc, mybir.MemoryLocationSet):
            continue
        name = alloc.memorylocations[0].name
        if alloc.kind == "ExternalInput":
            if name != partition_name:
                in_names.append(name)
        elif alloc.kind == "ExternalOutput":
            out_names.append(name)
            out_avals.append(jax.core.ShapedArray(
                tuple(alloc.tensor_shape), mybir.dt.np(alloc.dtype)))
    n_params = len(in_names)
    all_in_names = list(in_names) + list(out_names)
    if partition_name is not None:
        all_in_names.append(partition_name)

    def _body(*args):
        operands = list(args)
        if partition_name is not None:
            operands.append(partition_id_tensor())
        return tuple(_bass_exec_p.bind(
            *operands, out_avals=tuple(out_avals),
            in_names=tuple(all_in_names), out_names=tuple(out_names),
            lowering_input_output_aliases=(), sim_require_finite=True,
            sim_require_nnan=True, nc=nc))

    devices = jax.devices()[:NCORES]
    assert len(devices) == NCORES, f"need {NCORES} devices, got {devices}"
    mesh = Mesh(np.asarray(devices), ("core",))
    spec = PartitionSpec("core")
    fn = jax.jit(
        shard_map(_body, mesh=mesh, in_specs=(spec,) * (n_params + len(out_names)),
                  out_specs=(spec,) * len(out_names), check_rep=False),
        keep_unused=True)
    sh = NamedSharding(mesh, spec)
    dev_in = [
        jax.device_put(
            np.concatenate([np.asarray(in_maps[c][name])
                            for c in range(NCORES)], axis=0), sh)
        for name in in_names
    ] + [
        jax.device_put(
            np.zeros((NCORES * a.shape[0], *a.shape[1:]), a.dtype), sh)
        for a in out_avals
    ]
    outs = fn(*dev_in)
    jax.block_until_ready(outs)
    return [
        {name: np.asarray(outs[i]).reshape(NCORES, *out_avals[i].shape)[c]
         for i, name in enumerate(out_names)}
        for c in range(NCORES)
    ]


def kernel(atom_fea: np.ndarray, segment_ids: np.ndarray,
           num_crystals=N0) -> np.ndarray:
    assert int(num_crystals) == N0
    assert atom_fea.shape == (N, FEA)
    in_maps, plan = prepare_inputs(atom_fea, segment_ids)
    nc = build_program(plan)
    if _HAVE_FAST_PATH:
        try:
            return assemble_output(_run_spmd_fast(nc, in_maps))
        except Exception:
            pass
    res = run_bass_kernel_spmd(nc, in_maps, list(range(NCORES)))
    return assemble_output(res.results)


# revision 49
# speedup vs baseline: 1.8864x; 1.8864x over previous
"""Segment-mean (CGCNN crystal pooling) Bass kernel for 8 Trainium2 NeuronCores.

Reference computes, for sorted segment_ids over 1M atoms with 128 features:
    out[s] = sum(atom_fea[segment_ids == s]) / max(count(s), 1)   s in [0, 16384)

Strategy (data-parallel over crystals, no cross-device communication):
  - Core c owns segments [2048*c, 2048*(c+1)) = G groups of W segments.
  - Host pads each group's atoms to a uniform budget T*128 and lays them out
    partition-major: column block t of fea[g] ([128, T*128]) holds atom tile t
    ([128 atoms in partitions] x [128 features]).
  - Features ship as a SINGLE bf16 stream (harness gate is rel_err < 2e-2;
    bf16 rounding of the inputs gives ~2e-3 through the mean) -> HBM traffic
    is halved vs an exact hi/lo pair.
  - Narrow-window matmuls: segment_ids are sorted, so the 128 atoms of tile t
    only span a few segments. b(t) = min over ALL 128 groups (8 cores x G --
    the SPMD program is shared) of the group-relative id at atom position
    128*t, clamped to W-S; S = max span observed. Tile t's matmul then writes
    only psum[:, b(t):b(t)+S] with a [128, S] one-hot slice as the moving
    operand. A single full-width zero matmul per group initializes PSUM.
  - The one-hot block [128, T*S] is ONE DVE is_equal per group: a host-shipped
    tiled int8 iota constant vs a stride-0 broadcast of per-atom int8 ids
    relative to b(t) (padding atoms carry -1 and zero features). ~12x less
    DVE work than a full-width [128, T*W] compare.
  - Divide-by-count on device: a K=1 matmul broadcasts the group's 1/count
    row (f32) into all 128 PSUM partitions, ACT stages it to SBUF, and the
    eviction is a DVE multiply psum * invc -> bf16 out (the harness gate is
    2e-2; bf16 out adds ~1e-3).

Measured on trn2 (8 cores, axon): ~90-104 us/kernel (best 89.8, session
noise is +-8%) against a ~96 us DMA wire floor (34.6 MB/core at 16 engines
x 22.5 B/ns nominal; good sessions sustain ~385 GB/s/core). Baseline (exact
bf16 hi/lo pair, full-width one-hot, f32 out): ~216 us. Max relative error
vs the f32 reference: 3.1e-03 (gate: 2e-2).
"""

import contextlib

import ml_dtypes
import numpy as np

import concourse.bass as bass
import concourse.tile as tile
from concourse import bacc, mybir
from concourse.bass_utils import run_bass_kernel_spmd

try:
    import jax
    from jax.experimental.shard_map import shard_map
    from jax.sharding import Mesh, NamedSharding, PartitionSpec
    from concourse.bass2jax import (_bass_exec_p, install_neuronx_cc_hook,
                                    partition_id_tensor)
    _HAVE_FAST_PATH = True
except Exception:  # pragma: no cover - fall back to run_bass_kernel_spmd
    _HAVE_FAST_PATH = False

N = 1048576
FEA = 128
N0 = 16384
NCORES = 8
W = 512                     # segments per group (PSUM free dim = full bank)
SEGS_PER_CORE = N0 // NCORES  # 2048
G = SEGS_PER_CORE // W      # groups per core
NGROUPS = N0 // W           # groups total (all cores share one SPMD program)
P = 128
SB = 37                     # atom tiles per fea DMA block
FEA_BUFS = 5
DUAL_RING = True            # alternate fea slabs between sync and ACT rings
BF16 = ml_dtypes.bfloat16

_prog_cache: dict = {}


def build_program(plan, loop_repeat: int = 1):
    """SPMD Tile program for plan = (T, S, bases).

    T atom-tiles (T*128 atoms) per group; tile t's matmul writes the S-wide
    psum window starting at compile-time base bases[t]. loop_repeat > 1 wraps
    the body in a hardware For_i loop (timing only)."""
    T, S, bases = plan
    key = (T, S, bases, loop_repeat, SB, FEA_BUFS, DUAL_RING)
    if key in _prog_cache:
        return _prog_cache[key]

    f32 = mybir.dt.float32
    bf16 = mybir.dt.bfloat16
    nc = bacc.Bacc("TRN2", target_bir_lowering=False, debug=False,
                   num_devices=NCORES)
    i8 = mybir.dt.int8
    fhi = nc.dram_tensor("fhi", [G, P, T * P], bf16, kind="ExternalInput").ap()
    idsr = nc.dram_tensor("idsr", [P, G * T], i8, kind="ExternalInput").ap()
    iotar = nc.dram_tensor("iotar", [1, T * S], i8, kind="ExternalInput").ap()
    invc = nc.dram_tensor("invc", [1, G * W], f32, kind="ExternalInput").ap()
    out = nc.dram_tensor("out", [G, P, W], bf16, kind="ExternalOutput").ap()

    sb = min(T, SB)
    blocks = [(s, min(s + sb, T)) for s in range(0, T, sb)]

    with tile.TileContext(nc) as tc:
        with (
            tc.tile_pool(name="const", bufs=1) as const_pool,
            tc.tile_pool(name="fea", bufs=FEA_BUFS) as fea_pool,
            tc.tile_pool(name="meta", bufs=3) as meta_pool,
            tc.tile_pool(name="oh", bufs=2) as oh_pool,
            tc.tile_pool(name="evict", bufs=2) as evict_pool,
            tc.tile_pool(name="psum", bufs=2, space="PSUM") as psum_pool,
            tc.tile_pool(name="invp", bufs=2, space="PSUM") as invp_pool,
        ):
            # one 4.6KB DRAM row replicated across all 128 partitions
            iota_rep = const_pool.tile([P, T * S], i8)
            nc.scalar.dma_start(iota_rep[:], iotar.to_broadcast([P, T * S]))
            zeros = const_pool.tile([P, max(W, P)], bf16)
            nc.vector.memset(zeros[:], 0.0)
            ones = const_pool.tile([1, P], f32)
            nc.vector.memset(ones[:], 1.0)

            loop_ctx = (tc.For_i(0, loop_repeat, 1) if loop_repeat > 1
                        else contextlib.nullcontext())
            with loop_ctx:
                # all groups' meta in ONE wide DMA each (>=1KB rows dodge the
                # sub-512B small-transfer penalty); bulk fea on the sync queue
                ids_all = meta_pool.tile([P, G * T], i8, tag="ids")
                nc.scalar.dma_start(ids_all[:], idsr)
                invc_all = meta_pool.tile([1, G * W], f32, tag="invc")
                nc.scalar.dma_start(invc_all[:], invc)
                for g in range(G):
                    # K=1 matmul broadcasts 1/count into all 128 partitions
                    # (then ACT stages it to SBUF: DVE has one PSUM read port)
                    invp = invp_pool.tile([P, W], f32)
                    nc.tensor.matmul(out=invp[:], lhsT=ones[:],
                                     rhs=invc_all[:, g * W:(g + 1) * W],
                                     start=True, stop=True)
                    invp_sb = meta_pool.tile([P, W], f32)
                    nc.scalar.copy(invp_sb[:], invp[:])
                    oh_blk = oh_pool.tile([P, T * S], bf16)
                    nc.vector.tensor_tensor(
                        out=oh_blk[:], in0=iota_rep[:],
                        in1=ids_all[:, g * T:(g + 1) * T].to_broadcast(
                            [P, T, S]),
                        op=mybir.AluOpType.is_equal)
                    psum = psum_pool.tile([P, W], f32)
                    nc.tensor.matmul(
                        out=psum[:], lhsT=zeros[:, :P], rhs=zeros[:, :W],
                        start=True, stop=False)
                    for bi, (s, e) in enumerate(blocks):
                        hi_sb = fea_pool.tile([P, sb * P], bf16, tag="hi")
                        eng = (nc.scalar if (DUAL_RING and bi % 2)
                               else nc.sync)
                        eng.dma_start(hi_sb[:, :(e - s) * P],
                                      fhi[g][:, s * P:e * P])
                        for t in range(s, e):
                            c0 = (t - s) * P
                            b = bases[t]
                            nc.tensor.matmul(
                                out=psum[:, b:b + S],
                                lhsT=hi_sb[:, c0:c0 + P],
                                rhs=oh_blk[:, t * S:(t + 1) * S],
                                start=False, stop=(t == T - 1))
                    out_sb = evict_pool.tile([P, W], bf16)
                    nc.vector.tensor_tensor(out=out_sb[:], in0=psum[:],
                                            in1=invp_sb[:],
                                            op=mybir.AluOpType.mult)
                    nc.scalar.dma_start(out[g], out_sb[:])
    nc.compile()
    _prog_cache[key] = nc
    return nc


def prepare_inputs(atom_fea: np.ndarray, segment_ids: np.ndarray):
    """Shard + pad + layout inputs for the 8 cores. Returns (in_maps, plan)."""
    atom_fea = np.ascontiguousarray(atom_fea, dtype=np.float32)
    segment_ids = np.ascontiguousarray(segment_ids, dtype=np.int32)

    counts = np.bincount(segment_ids, minlength=N0)
    inv_counts = (1.0 / np.maximum(counts, 1)).astype(np.float32)

    bounds = np.searchsorted(segment_ids, np.arange(0, N0 + 1, W))
    ng = np.diff(bounds)
    T = max(1, int(np.ceil(ng.max() / P)))

    # Narrow-window plan shared by the single SPMD program: for tile t,
    # b(t) = min over all groups of the group-relative id of atom 128*t,
    # S = max observed span (id - b + 1) within any tile.
    lo_t = np.full(T, np.iinfo(np.int64).max, dtype=np.int64)
    hi_t = np.full(T, -1, dtype=np.int64)
    rel_groups = []
    for gi in range(NGROUPS):
        a = segment_ids[bounds[gi]:bounds[gi + 1]].astype(np.int64) - W * gi
        rel_groups.append(a)
        nt = int(np.ceil(len(a) / P))
        for t in range(nt):
            seg = a[t * P:(t + 1) * P]
            lo_t[t] = min(lo_t[t], seg[0])
            hi_t[t] = max(hi_t[t], seg[-1])
    S = int((hi_t - np.minimum(lo_t, hi_t + 1) + 1).max())
    bases = np.minimum(np.minimum(lo_t, W - S), hi_t + 1)
    bases = np.maximum(bases, 0)
    # guarantee every tile's ids fall inside [b, b+S) for every group
    assert int((hi_t - bases + 1).max()) <= S
    plan = (T, S, tuple(int(x) for x in bases))

    hi_full = atom_fea.astype(BF16)

    iota_rep = np.tile(np.arange(S, dtype=np.int8), T).reshape(1, T * S)

    in_maps = []
    for c in range(NCORES):
        hi_c = np.zeros((G, P, T * P), dtype=BF16)
        ids_c = np.empty((P, G * T), dtype=np.int8)
        for g in range(G):
            gidx = c * G + g
            lo_i, hi_i = bounds[gidx], bounds[gidx + 1]
            n = hi_i - lo_i
            blk = np.zeros((T * P, FEA), dtype=BF16)
            blk[:n] = hi_full[lo_i:hi_i]
            hi_c[g] = blk.reshape(T, P, FEA).transpose(1, 0, 2).reshape(
                P, T * P)
            idb = np.full(T * P, -1, dtype=np.int64)
            idb[:n] = rel_groups[gidx] - np.repeat(bases, P)[:n]
            ids_c[:, g * T:(g + 1) * T] = idb.reshape(T, P).T.astype(np.int8)
        invc_c = inv_counts[c * SEGS_PER_CORE:(c + 1) * SEGS_PER_CORE].reshape(
            1, G * W)
        in_maps.append({"fhi": hi_c, "idsr": ids_c, "iotar": iota_rep,
                        "invc": invc_c})
    return in_maps, plan


def assemble_output(results) -> np.ndarray:
    """[ncores][G, 128 fea, W seg] -> (N0, FEA)."""
    stacked = np.stack([results[c]["out"] for c in range(NCORES)]).astype(
        np.float32)
    return np.ascontiguousarray(
        stacked.transpose(0, 1, 3, 2).reshape(N0, FEA))


def _run_spmd_fast(nc, in_maps):
    """Execute the SPMD program on cores 0-7 via PJRT with explicit sharded
    device_put (same _bass_exec_p mechanism run_bass_kernel_spmd uses under
    axon, minus its per-call retrace and slow implicit transfers)."""
    install_neuronx_cc_hook()
    partition_name = (nc.partition_id_tensor.name
                      if nc.partition_id_tensor else None)
    in_names, out_names, out_avals = [], [], []
    for alloc in nc.m.functions[0].allocations:
        if not isinstance(alloc, mybir.MemoryLocationSet):
            continue
        name = alloc.memorylocations[0].name
        if alloc.kind == "ExternalInput":
            if name != partition_name:
                in_names.append(name)
        elif alloc.kind == "ExternalOutput":
            out_names.append(name)
            out_avals.append(jax.core.ShapedArray(
                tuple(alloc.tensor_shape), mybir.dt.np(alloc.dtype)))
    n_params = len(in_names)
    all_in_names = list(in_names) + list(out_names)
    if partition_name is not None:
        all_in_names.append(partition_name)

    def _body(*args):
        operands = list(args)
        if partition_name is not None:
            operands.append(partition_id_tensor())
        return tuple(_bass_exec_p.bind(
            *operands, out_avals=tuple(out_avals),
            in_names=tuple(all_in_names), out_names=tuple(out_names),
            lowering_input_output_aliases=(), sim_require_finite=True,
            sim_require_nnan=True, nc=nc))

    devices = jax.devices()[:NCORES]
    assert len(devices) == NCORES, f"need {NCORES} devices, got {devices}"
    mesh = Mesh(np.asarray(devices), ("core",))
    spec = PartitionSpec("core")
    fn = jax.jit(
        shard_map(_body, mesh=mesh, in_specs=(spec,) * (n_params + len(out_names)),
                  out_specs=(spec,) * len(out_names), check_rep=False),
        keep_unused=True)
    sh = NamedSharding(mesh, spec)
    dev_in = [
        jax.device_put(
            np.concatenate([np.asarray(in_maps[c][name])
                            for c in range(NCORES)], axis=0), sh)
        for name in in_names
    ] + [
        jax.device_put(
            np.zeros((NCORES * a.shape[0], *a.shape[1:]), a.dtype), sh)
        for a in out_avals
    ]
    outs = fn(*dev_in)
    jax.block_until_ready(outs)
    return [
        {name: np.asarray(outs[i]).reshape(NCORES, *out_avals[i].shape)[c]
         for i, name in enumerate(out_names)}
        for c in range(NCORES)
    ]


def kernel(atom_fea: np.ndarray, segment_ids: np.ndarray,
           num_crystals=N0) -> np.ndarray:
    assert int(num_crystals) == N0
    assert atom_fea.shape == (N, FEA)
    in_maps, plan = prepare_inputs(atom_fea, segment_ids)
    nc = build_program(plan)
    if _HAVE_FAST_PATH:
        try:
            return assemble_output(_run_spmd_fast(nc, in_maps))
        except Exception:
            pass
    res = run_bass_kernel_spmd(nc, in_maps, list(range(NCORES)))
    return assemble_output(res.results)


# revision 50
# speedup vs baseline: 2.0762x; 1.1007x over previous
"""Segment-mean (CGCNN crystal pooling) Bass kernel for 8 Trainium2 NeuronCores.

Reference computes, for sorted segment_ids over 1M atoms with 128 features:
    out[s] = sum(atom_fea[segment_ids == s]) / max(count(s), 1)   s in [0, 16384)

Strategy (data-parallel over crystals, no cross-device communication):
  - Core c owns segments [2048*c, 2048*(c+1)) = G groups of W segments.
  - Host pads each group's atoms to a uniform budget T*128 and lays them out
    partition-major: column block t of fea[g] ([128, T*128]) holds atom tile t
    ([128 atoms in partitions] x [128 features]).
  - Features ship as a SINGLE bf16 stream (harness gate is rel_err < 2e-2;
    bf16 rounding of the inputs gives ~2e-3 through the mean) -> HBM traffic
    is halved vs an exact hi/lo pair.
  - Narrow-window matmuls: segment_ids are sorted, so the 128 atoms of tile t
    only span a few segments. b(t) = min over ALL 128 groups (8 cores x G --
    the SPMD program is shared) of the group-relative id at atom position
    128*t, clamped to W-S; S = max span observed. Tile t's matmul then writes
    only psum[:, b(t):b(t)+S] with a [128, S] one-hot slice as the moving
    operand. A single full-width zero matmul per group initializes PSUM.
  - The one-hot block [128, T*S] is ONE DVE is_equal per group: a host-shipped
    tiled int8 iota constant vs a stride-0 broadcast of per-atom int8 ids
    relative to b(t) (padding atoms carry -1 and zero features). ~12x less
    DVE work than a full-width [128, T*W] compare.
  - Divide-by-count on device: a K=1 matmul broadcasts the group's 1/count
    row (f32) into all 128 PSUM partitions, ACT stages it to SBUF, and the
    eviction is a DVE multiply psum * invc -> bf16 out (the harness gate is
    2e-2; bf16 out adds ~1e-3).

Measured on trn2 (8 cores, axon): ~90-104 us/kernel (best 89.8, session
noise is +-8%) against a ~96 us DMA wire floor (34.6 MB/core at 16 engines
x 22.5 B/ns nominal; good sessions sustain ~385 GB/s/core). Baseline (exact
bf16 hi/lo pair, full-width one-hot, f32 out): ~216 us. Max relative error
vs the f32 reference: 3.1e-03 (gate: 2e-2).
"""

import contextlib

import ml_dtypes
import numpy as np

import concourse.bass as bass
import concourse.tile as tile
from concourse import bacc, mybir
from concourse.bass_utils import run_bass_kernel_spmd

try:
    import jax
    from jax.experimental.shard_map import shard_map
    from jax.sharding import Mesh, NamedSharding, PartitionSpec
    from concourse.bass2jax import (_bass_exec_p, install_neuronx_cc_hook,
                                    partition_id_tensor)
    _HAVE_FAST_PATH = True
except Exception:  # pragma: no cover - fall back to run_bass_kernel_spmd
    _HAVE_FAST_PATH = False

N = 1048576
FEA = 128
N0 = 16384
NCORES = 8
W = 512                     # segments per group (PSUM free dim = full bank)
SEGS_PER_CORE = N0 // NCORES  # 2048
G = SEGS_PER_CORE // W      # groups per core
NGROUPS = N0 // W           # groups total (all cores share one SPMD program)
P = 128
SB = 37                     # atom tiles per fea DMA block
FEA_BUFS = 5
DUAL_RING = False           # alternate fea slabs between sync and ACT rings
BF16 = ml_dtypes.bfloat16

_prog_cache: dict = {}


def build_program(plan, loop_repeat: int = 1):
    """SPMD Tile program for plan = (T, S, bases).

    T atom-tiles (T*128 atoms) per group; tile t's matmul writes the S-wide
    psum window starting at compile-time base bases[t]. loop_repeat > 1 wraps
    the body in a hardware For_i loop (timing only)."""
    T, S, bases = plan
    key = (T, S, bases, loop_repeat, SB, FEA_BUFS, DUAL_RING)
    if key in _prog_cache:
        return _prog_cache[key]

    f32 = mybir.dt.float32
    bf16 = mybir.dt.bfloat16
    nc = bacc.Bacc("TRN2", target_bir_lowering=False, debug=False,
                   num_devices=NCORES)
    i8 = mybir.dt.int8
    fhi = nc.dram_tensor("fhi", [G, P, T * P], bf16, kind="ExternalInput").ap()
    idsr = nc.dram_tensor("idsr", [P, G * T], i8, kind="ExternalInput").ap()
    iotar = nc.dram_tensor("iotar", [1, T * S], i8, kind="ExternalInput").ap()
    invc = nc.dram_tensor("invc", [1, G * W], f32, kind="ExternalInput").ap()
    out = nc.dram_tensor("out", [G, P, W], bf16, kind="ExternalOutput").ap()

    sb = min(T, SB)
    blocks = [(s, min(s + sb, T)) for s in range(0, T, sb)]

    with tile.TileContext(nc) as tc:
        with (
            tc.tile_pool(name="const", bufs=1) as const_pool,
            tc.tile_pool(name="fea", bufs=FEA_BUFS) as fea_pool,
            tc.tile_pool(name="meta", bufs=3) as meta_pool,
            tc.tile_pool(name="oh", bufs=2) as oh_pool,
            tc.tile_pool(name="evict", bufs=2) as evict_pool,
            tc.tile_pool(name="psum", bufs=2, space="PSUM") as psum_pool,
            tc.tile_pool(name="invp", bufs=2, space="PSUM") as invp_pool,
        ):
            # one 4.6KB DRAM row replicated across all 128 partitions
            iota_rep = const_pool.tile([P, T * S], i8)
            nc.scalar.dma_start(iota_rep[:], iotar.to_broadcast([P, T * S]))
            zeros = const_pool.tile([P, max(W, P)], bf16)
            nc.vector.memset(zeros[:], 0.0)
            ones = const_pool.tile([1, P], f32)
            nc.vector.memset(ones[:], 1.0)

            loop_ctx = (tc.For_i(0, loop_repeat, 1) if loop_repeat > 1
                        else contextlib.nullcontext())
            with loop_ctx:
                # all groups' meta in ONE wide DMA each (>=1KB rows dodge the
                # sub-512B small-transfer penalty); bulk fea on the sync queue
                ids_all = meta_pool.tile([P, G * T], i8, tag="ids")
                nc.scalar.dma_start(ids_all[:], idsr)
                invc_all = meta_pool.tile([1, G * W], f32, tag="invc")
                nc.scalar.dma_start(invc_all[:], invc)
                for g in range(G):
                    # K=1 matmul broadcasts 1/count into all 128 partitions
                    # (then ACT stages it to SBUF: DVE has one PSUM read port)
                    invp = invp_pool.tile([P, W], f32)
                    nc.tensor.matmul(out=invp[:], lhsT=ones[:],
                                     rhs=invc_all[:, g * W:(g + 1) * W],
                                     start=True, stop=True)
                    invp_sb = meta_pool.tile([P, W], f32)
                    nc.scalar.copy(invp_sb[:], invp[:])
                    oh_blk = oh_pool.tile([P, T * S], bf16)
                    nc.vector.tensor_tensor(
                        out=oh_blk[:], in0=iota_rep[:],
                        in1=ids_all[:, g * T:(g + 1) * T].to_broadcast(
                            [P, T, S]),
                        op=mybir.AluOpType.is_equal)
                    psum = psum_pool.tile([P, W], f32)
                    nc.tensor.matmul(
                        out=psum[:], lhsT=zeros[:, :P], rhs=zeros[:, :W],
                        start=True, stop=False)
                    for bi, (s, e) in enumerate(blocks):
                        hi_sb = fea_pool.tile([P, sb * P], bf16, tag="hi")
                        eng = (nc.scalar if (DUAL_RING and bi % 2)
                               else nc.sync)
                        eng.dma_start(hi_sb[:, :(e - s) * P],
                                      fhi[g][:, s * P:e * P])
                        for t in range(s, e):
                            c0 = (t - s) * P
                            b = bases[t]
                            nc.tensor.matmul(
                                out=psum[:, b:b + S],
                                lhsT=hi_sb[:, c0:c0 + P],
                                rhs=oh_blk[:, t * S:(t + 1) * S],
                                start=False, stop=(t == T - 1))
                    out_sb = evict_pool.tile([P, W], bf16)
                    nc.vector.tensor_tensor(out=out_sb[:], in0=psum[:],
                                            in1=invp_sb[:],
                                            op=mybir.AluOpType.mult)
                    nc.scalar.dma_start(out[g], out_sb[:])
    nc.compile()
    _prog_cache[key] = nc
    return nc


def prepare_inputs(atom_fea: np.ndarray, segment_ids: np.ndarray):
    """Shard + pad + layout inputs for the 8 cores. Returns (in_maps, plan)."""
    atom_fea = np.ascontiguousarray(atom_fea, dtype=np.float32)
    segment_ids = np.ascontiguousarray(segment_ids, dtype=np.int32)

    counts = np.bincount(segment_ids, minlength=N0)
    inv_counts = (1.0 / np.maximum(counts, 1)).astype(np.float32)

    bounds = np.searchsorted(segment_ids, np.arange(0, N0 + 1, W))
    ng = np.diff(bounds)
    T = max(1, int(np.ceil(ng.max() / P)))

    # Narrow-window plan shared by the single SPMD program: for tile t,
    # b(t) = min over all groups of the group-relative id of atom 128*t,
    # S = max observed span (id - b + 1) within any tile.
    lo_t = np.full(T, np.iinfo(np.int64).max, dtype=np.int64)
    hi_t = np.full(T, -1, dtype=np.int64)
    rel_groups = []
    for gi in range(NGROUPS):
        a = segment_ids[bounds[gi]:bounds[gi + 1]].astype(np.int64) - W * gi
        rel_groups.append(a)
        nt = int(np.ceil(len(a) / P))
        for t in range(nt):
            seg = a[t * P:(t + 1) * P]
            lo_t[t] = min(lo_t[t], seg[0])
            hi_t[t] = max(hi_t[t], seg[-1])
    S = int((hi_t - np.minimum(lo_t, hi_t + 1) + 1).max())
    bases = np.minimum(np.minimum(lo_t, W - S), hi_t + 1)
    bases = np.maximum(bases, 0)
    # guarantee every tile's ids fall inside [b, b+S) for every group
    assert int((hi_t - bases + 1).max()) <= S
    plan = (T, S, tuple(int(x) for x in bases))

    hi_full = atom_fea.astype(BF16)

    iota_rep = np.tile(np.arange(S, dtype=np.int8), T).reshape(1, T * S)

    in_maps = []
    for c in range(NCORES):
        hi_c = np.zeros((G, P, T * P), dtype=BF16)
        ids_c = np.empty((P, G * T), dtype=np.int8)
        for g in range(G):
            gidx = c * G + g
            lo_i, hi_i = bounds[gidx], bounds[gidx + 1]
            n = hi_i - lo_i
            blk = np.zeros((T * P, FEA), dtype=BF16)
            blk[:n] = hi_full[lo_i:hi_i]
            hi_c[g] = blk.reshape(T, P, FEA).transpose(1, 0, 2).reshape(
                P, T * P)
            idb = np.full(T * P, -1, dtype=np.int64)
            idb[:n] = rel_groups[gidx] - np.repeat(bases, P)[:n]
            ids_c[:, g * T:(g + 1) * T] = idb.reshape(T, P).T.astype(np.int8)
        invc_c = inv_counts[c * SEGS_PER_CORE:(c + 1) * SEGS_PER_CORE].reshape(
            1, G * W)
        in_maps.append({"fhi": hi_c, "idsr": ids_c, "iotar": iota_rep,
                        "invc": invc_c})
    return in_maps, plan


def assemble_output(results) -> np.ndarray:
    """[ncores][G, 128 fea, W seg] -> (N0, FEA)."""
    stacked = np.stack([results[c]["out"] for c in range(NCORES)]).astype(
        np.float32)
    return np.ascontiguousarray(
        stacked.transpose(0, 1, 3, 2).reshape(N0, FEA))


def _run_spmd_fast(nc, in_maps):
    """Execute the SPMD program on cores 0-7 via PJRT with explicit sharded
    device_put (same _bass_exec_p mechanism run_bass_kernel_spmd uses under
    axon, minus its per-call retrace and slow implicit transfers)."""
    install_neuronx_cc_hook()
    partition_name = (nc.partition_id_tensor.name
                      if nc.partition_id_tensor else None)
    in_names, out_names, out_avals = [], [], []
    for alloc in nc.m.functions[0].allocations:
        if not isinstance(alloc, mybir.MemoryLocationSet):
            continue
        name = alloc.memorylocations[0].name
        if alloc.kind == "ExternalInput":
            if name != partition_name:
                in_names.append(name)
        elif alloc.kind == "ExternalOutput":
            out_names.append(name)
            out_avals.append(jax.core.ShapedArray(
                tuple(alloc.tensor_shape), mybir.dt.np(alloc.dtype)))
    n_params = len(in_names)
    all_in_names = list(in_names) + list(out_names)
    if partition_name is not None:
        all_in_names.append(partition_name)

    def _body(*args):
        operands = list(args)
        if partition_name is not None:
            operands.append(partition_id_tensor())
        return tuple(_bass_exec_p.bind(
            *operands, out_avals=tuple(out_avals),
            in_names=tuple(all_in_names), out_names=tuple(out_names),
            lowering_input_output_aliases=(), sim_require_finite=True,
            sim_require_nnan=True, nc=nc))

    devices = jax.devices()[:NCORES]
    assert len(devices) == NCORES, f"need {NCORES} devices, got {devices}"
    mesh = Mesh(np.asarray(devices), ("core",))
    spec = PartitionSpec("core")
    fn = jax.jit(
        shard_map(_body, mesh=mesh, in_specs=(spec,) * (n_params + len(out_names)),
                  out_specs=(spec,) * len(out_names), check_rep=False),
        keep_unused=True)
    sh = NamedSharding(mesh, spec)
    dev_in = [
        jax.device_put(
            np.concatenate([np.asarray(in_maps[c][name])
                            for c in range(NCORES)], axis=0), sh)
        for name in in_names
    ] + [
        jax.device_put(
            np.zeros((NCORES * a.shape[0], *a.shape[1:]), a.dtype), sh)
        for a in out_avals
    ]
    outs = fn(*dev_in)
    jax.block_until_ready(outs)
    return [
        {name: np.asarray(outs[i]).reshape(NCORES, *out_avals[i].shape)[c]
         for i, name in enumerate(out_names)}
        for c in range(NCORES)
    ]


def kernel(atom_fea: np.ndarray, segment_ids: np.ndarray,
           num_crystals=N0) -> np.ndarray:
    assert int(num_crystals) == N0
    assert atom_fea.shape == (N, FEA)
    in_maps, plan = prepare_inputs(atom_fea, segment_ids)
    nc = build_program(plan)
    if _HAVE_FAST_PATH:
        try:
            return assemble_output(_run_spmd_fast(nc, in_maps))
        except Exception:
            pass
    res = run_bass_kernel_spmd(nc, in_maps, list(range(NCORES)))
    return assemble_output(res.results)
